# revision 1
# baseline (speedup 1.0000x reference)
"""Trainium2 Bass kernel: colorization via Jacobi color propagation.

Algorithm (mirrors the reference):
  - per-pixel 8-neighbor affinity weights from local luminance variance
  - x <- b + W x Jacobi iterations (100) on the 2 chroma channels
  - output = yiq2rgb(Y, x)

Distribution: image split into 8 row-strips (128 rows/core).  Each core
keeps its strip in SBUF for the entire run.  Layout per core puts image
COLUMNS on SBUF partitions (9 groups of 126 owned columns + 2 guard
partitions that mirror the neighboring groups' edge columns) and ROWS in
the free dimension.  Time-batched halo exchange: each core carries T
ghost rows on each side of its strip (in the free dim, so they cost only
(128+2T)/128 extra work) and re-syncs ghosts with an 8-core AllGather
every T iterations.

Per Jacobi iteration (all flat, partition-aligned access patterns):
  - VectorE: 8 fp16 tensor-tensor multiplies Q_k = w~_k * shift_rows(x)
    (w~_k are the affinity weights pre-shifted along the column/partition
    axis at setup time so the hot loop never crosses partitions)
  - TensorE: 9-term accumulation into PSUM via shift-matrix matmuls
    (the stationary matrix applies the +-1 column shift for free)
  - ScalarE: evacuate PSUM -> SBUF x (fp32 -> fp16 cast)
  - tiny SBUF->SBUF DMAs refresh the guard partitions
"""
import sys

sys.path.insert(0, "/opt/trn_rl_repo")

from dataclasses import dataclass

import numpy as np

import concourse.bass as bass
import concourse.bacc as bacc
import concourse.mybir as mybir
from concourse import tile

F32 = mybir.dt.float32
I32 = mybir.dt.int32

OFFSETS = [(-1, -1), (-1, 0), (-1, 1), (0, -1), (0, 1), (1, -1), (1, 0), (1, 1)]
# dy -> stationary matrix index (0: identity, 1: out[p]=Q[p+1], 2: out[p]=Q[p-1])
MAT_IDX = {0: 0, 1: 1, -1: 2}

YIQ2RGB = [
    [1.0, 0.9468822170900693, 0.6235565819861433],
    [1.0, -0.27478764629897834, -0.6356910791873801],
    [1.0, -1.1085450346420322, 1.7090069284064666],
]


@dataclass(frozen=True)
class Params:
    H: int = 1024
    W: int = 1024
    ncores: int = 8
    n_iters: int = 100
    T: int = 8          # ghost depth (iterations between halo exchanges)
    cpg: int = 126      # owned columns per partition-group
    ns: int = 2         # column-group sets per iteration (pipeline granularity)
    fp16: bool = True   # iterate in fp16 (fp32 used for debugging)

    @property
    def rpc(self):  # rows per core
        return self.H // self.ncores

    @property
    def R(self):  # local rows incl. T ghosts each side + 2 zero guard rows
        return self.rpc + 2 * self.T + 2

    @property
    def NG(self):  # column groups
        return -(-self.W // self.cpg)

    @property
    def F1(self):
        return self.NG * self.R

    @property
    def R2(self):
        return 2 * self.R

    @property
    def W2(self):
        return self.NG * self.R2

    @property
    def dt16(self):
        return mybir.dt.float16 if self.fp16 else mybir.dt.float32

    @property
    def np16(self):
        return np.float16 if self.fp16 else np.float32


PADE = 4  # fp16 flat-array padding (elements) on each side of x16


def _sets(p: Params):
    """Contiguous group ranges [(g0, g1), ...] for the ns pipeline sets."""
    base = p.NG // p.ns
    rem = p.NG % p.ns
    out = []
    g0 = 0
    for s in range(p.ns):
        g1 = g0 + base + (1 if s < rem else 0)
        out.append((g0, g1))
        g0 = g1
    return out


def _chunks(width32: int, cap: int = 512):
    """[(offset, size), ...] covering [0, width32) in <=cap pieces."""
    out = []
    o = 0
    while o < width32:
        out.append((o, min(cap, width32 - o)))
        o += cap
    return out


def build(p: Params):
    nc = bacc.Bacc("TRN2", target_bir_lowering=False, debug=False, num_devices=p.ncores)
    NG, R, R2, F1, W2 = p.NG, p.R, p.R2, p.F1, p.W2
    RPC, T = p.rpc, p.T
    dt16 = p.dt16

    gray_d = nc.dram_tensor("gray", [NG, 128, R, 3], F32, kind="ExternalInput")
    appx_d = nc.dram_tensor("appx", [NG, 128, R, 3], F32, kind="ExternalInput")
    vmask_d = nc.dram_tensor("vmask", [NG, 128, R], F32, kind="ExternalInput")
    mats_d = nc.dram_tensor("mats", [3, 128, 128], dt16, kind="ExternalInput")
    uhot_d = nc.dram_tensor("uhot", [128, 16], F32, kind="ExternalInput")
    out_d = nc.dram_tensor("out", [128, NG, RPC, 3], F32, kind="ExternalOutput")

    sets = _sets(p)

    with tile.TileContext(nc) as tc:
        with (
            tc.tile_pool(name="persist", bufs=1) as pers,
            tc.tile_pool(name="dram", bufs=1, space="DRAM") as dram,
        ):
            y32 = pers.tile([128, NG, R], F32)
            x16 = pers.tile([128, W2 + 2 * PADE], dt16)
            b16 = pers.tile([128, W2 + 2 * PADE], dt16)
            wde = [pers.tile([128, W2], dt16, name=f"wde{k}", tag=f"wde{k}") for k in range(8)]
            mats = pers.tile([128, 3, 128], dt16)
            uhot = pers.tile([128, 16], F32)
            xg_sb = pers.tile([128, p.ncores, 2, NG, T, 2], dt16)

            xbnd = dram.tile([128, 2, NG, T, 2], dt16)
            xgath = dram.tile([p.ncores, 128, 2, NG, T, 2], dt16)

            for i in range(3):
                nc.sync.dma_start(mats[:, i, :], mats_d[i])
            nc.sync.dma_start(uhot[:], uhot_d[:])

            # ---------------- setup: luma / chroma / colored mask ----------------
            with tc.tile_pool(name="mid", bufs=1) as mid:
                notc = mid.tile([128, NG, R], F32)

                with tc.tile_pool(name="ph1", bufs=1) as ph1:
                    g32 = ph1.tile([128, NG, R, 3], F32)
                    a32 = ph1.tile([128, NG, R, 3], F32)
                    for g in range(NG):
                        nc.sync.dma_start(g32[:, g], gray_d[g])
                        nc.sync.dma_start(a32[:, g], appx_d[g])

                    ya = ph1.tile([128, NG, R], F32)
                    t0 = ph1.tile([128, NG, R], F32)
                    t1 = ph1.tile([128, NG, R], F32)
                    t2 = ph1.tile([128, NG, R], F32)
                    s_abs = ph1.tile([128, NG, R], F32)
                    cmask = ph1.tile([128, NG, R], F32)

                    # y = (0.3 R + 0.59 G + 0.11 B)/255
                    for (src, dst) in ((g32, y32), (a32, ya)):
                        nc.vector.tensor_scalar_mul(t0[:], src[:, :, :, 0], 0.3 / 255.0)
                        nc.vector.scalar_tensor_tensor(
                            t0[:], src[:, :, :, 1], 0.59 / 255.0, t0[:],
                            mybir.AluOpType.mult, mybir.AluOpType.add)
                        nc.vector.scalar_tensor_tensor(
                            dst[:], src[:, :, :, 2], 0.11 / 255.0, t0[:],
                            mybir.AluOpType.mult, mybir.AluOpType.add)

                    # i = 0.74 (r-y) - 0.27 (b-y);  q = 0.48 (r-y) + 0.41 (b-y)  [appendix]
                    dr = ph1.tile([128, NG, R], F32)
                    db = ph1.tile([128, NG, R], F32)
                    nc.vector.scalar_tensor_tensor(
                        dr[:], a32[:, :, :, 0], 1.0 / 255.0, ya[:],
                        mybir.AluOpType.mult, mybir.AluOpType.subtract)
                    nc.vector.scalar_tensor_tensor(
                        db[:], a32[:, :, :, 2], 1.0 / 255.0, ya[:],
                        mybir.AluOpType.mult, mybir.AluOpType.subtract)
                    # s = sum |gray_c - appx_c|  (threshold 0.01*255 = 2.55)
                    nc.vector.tensor_sub(t1[:], g32[:, :, :, 0], a32[:, :, :, 0])
                    nc.scalar.activation(s_abs[:], t1[:], mybir.ActivationFunctionType.Abs)
                    for ch in (1, 2):
                        nc.vector.tensor_sub(t1[:], g32[:, :, :, ch], a32[:, :, :, ch])
                        nc.scalar.activation(t2[:], t1[:], mybir.ActivationFunctionType.Abs)
                        nc.vector.tensor_add(s_abs[:], s_abs[:], t2[:])
                    nc.vector.tensor_scalar(cmask[:], s_abs[:], 2.55, None, mybir.AluOpType.is_gt)
                    nc.vector.tensor_scalar(notc[:], s_abs[:], 2.55, None, mybir.AluOpType.is_le)

                    # b = isColored * IQ, stored fp16 ch-interleaved, guard rows zero
                    iA = ph1.tile([128, NG, R], F32)
                    qA = ph1.tile([128, NG, R], F32)
                    nc.vector.tensor_scalar_mul(t1[:], db[:], -0.27)
                    nc.vector.scalar_tensor_tensor(
                        iA[:], dr[:], 0.74, t1[:], mybir.AluOpType.mult, mybir.AluOpType.add)
                    nc.vector.tensor_scalar_mul(t1[:], db[:], 0.41)
                    nc.vector.scalar_tensor_tensor(
                        qA[:], dr[:], 0.48, t1[:], mybir.AluOpType.mult, mybir.AluOpType.add)
                    nc.vector.tensor_mul(iA[:], iA[:], cmask[:])
                    nc.vector.tensor_mul(qA[:], qA[:], cmask[:])

                    nc.vector.memset(b16[:], 0.0)
                    bview = b16[:, PADE : PADE + W2].rearrange(
                        "p (g r c) -> p g r c", g=NG, r=R, c=2)
                    nc.vector.tensor_copy(bview[:, :, 1 : R - 1, 0], iA[:, :, 1 : R - 1])
                    nc.vector.tensor_copy(bview[:, :, 1 : R - 1, 1], qA[:, :, 1 : R - 1])
                    nc.vector.memset(x16[:], 0.0)
                    nc.vector.tensor_copy(x16[:], b16[:])

                # ---------------- setup: affinity weights ----------------
                with tc.tile_pool(name="ph2", bufs=1) as ph2:
                    v32 = ph2.tile([128, NG, R], F32)
                    for g in range(NG):
                        nc.sync.dma_start(v32[:, g], vmask_d[g])

                    # partition-shifted planes (q+1 / q-1) of y and v
                    yp = ph2.tile([128, NG, R], F32)
                    ym = ph2.tile([128, NG, R], F32)
                    vp = ph2.tile([128, NG, R], F32)
                    vm = ph2.tile([128, NG, R], F32)
                    for t_ in (yp, ym, vp, vm):
                        nc.vector.memset(t_[:], 0.0)
                    nc.sync.dma_start(yp[0:127], y32[1:128])
                    nc.sync.dma_start(ym[1:128], y32[0:127])
                    nc.sync.dma_start(vp[0:127], v32[1:128])
                    nc.sync.dma_start(vm[1:128], v32[0:127])

                    ypl = {1: yp, 0: y32, -1: ym}
                    vpl = {1: vp, 0: v32, -1: vm}

                    def shifted(plane, dx):
                        return plane[:, :, 1 + dx : R - 1 + dx]

                    inner = lambda a: a[:, :, 1 : R - 1]

                    cnt = ph2.tile([128, NG, R], F32)
                    nbs = ph2.tile([128, NG, R], F32)
                    ssq = ph2.tile([128, NG, R], F32)
                    sc0 = ph2.tile([128, NG, R], F32)
                    sc1 = ph2.tile([128, NG, R], F32)
                    rcount = ph2.tile([128, NG, R], F32)

                    first = True
                    for dx, dy in OFFSETS:
                        if first:
                            nc.vector.tensor_copy(inner(cnt), shifted(vpl[dy], dx))
                            nc.vector.tensor_copy(inner(nbs), shifted(ypl[dy], dx))
                            nc.vector.tensor_mul(
                                inner(ssq), shifted(ypl[dy], dx), shifted(ypl[dy], dx))
                            first = False
                        else:
                            nc.vector.tensor_add(inner(cnt), inner(cnt), shifted(vpl[dy], dx))
                            nc.vector.tensor_add(inner(nbs), inner(nbs), shifted(ypl[dy], dx))
                            nc.vector.tensor_mul(
                                inner(sc0), shifted(ypl[dy], dx), shifted(ypl[dy], dx))
                            nc.vector.tensor_add(inner(ssq), inner(ssq), inner(sc0))

                    # count = cnt+1; mean = (nbs + y)/count
                    nc.vector.tensor_scalar_add(inner(sc0), inner(cnt), 1.0)
                    nc.vector.reciprocal(inner(rcount), inner(sc0))
                    mean = ph2.tile([128, NG, R], F32)
                    nc.vector.tensor_add(inner(sc0), inner(nbs), inner(y32))
                    nc.vector.tensor_mul(inner(mean), inner(sc0), inner(rcount))
                    # varnum = ssq - 2 mean nbs + mean^2 cnt + (y-mean)^2
                    var = ph2.tile([128, NG, R], F32)
                    nc.vector.tensor_mul(inner(sc0), inner(mean), inner(mean))
                    nc.vector.tensor_mul(inner(sc0), inner(sc0), inner(cnt))
                    nc.vector.tensor_mul(inner(sc1), inner(mean), inner(nbs))
                    nc.vector.scalar_tensor_tensor(
                        inner(sc1), inner(sc1), -2.0, inner(ssq),
                        mybir.AluOpType.mult, mybir.AluOpType.add)
                    nc.vector.tensor_add(inner(sc0), inner(sc0), inner(sc1))
                    nc.vector.tensor_sub(inner(sc1), inner(y32), inner(mean))
                    nc.vector.tensor_mul(inner(sc1), inner(sc1), inner(sc1))
                    nc.vector.tensor_add(inner(sc0), inner(sc0), inner(sc1))
                    nc.vector.tensor_mul(inner(var), inner(sc0), inner(rcount))
                    # negivs = -1 / max(0.6 var, 2e-6)
                    negivs = ph2.tile([128, NG, R], F32)
                    nc.vector.tensor_scalar(
                        inner(sc0), inner(var), 0.6, 2e-6,
                        mybir.AluOpType.mult, mybir.AluOpType.max)
                    nc.vector.reciprocal(inner(sc1), inner(sc0))
                    nc.vector.tensor_scalar_mul(inner(negivs), inner(sc1), -1.0)

                    # per-tap masked exp weights + wsum
                    wsum = ph2.tile([128, NG, R], F32)
                    mk = [ph2.tile([128, NG, R], F32, name=f"mk{k}", tag=f"mk{k}") for k in range(8)]
                    for k, (dx, dy) in enumerate(OFFSETS):
                        nc.vector.tensor_sub(inner(sc0), shifted(ypl[dy], dx), inner(y32))
                        nc.vector.tensor_mul(inner(sc0), inner(sc0), inner(sc0))
                        nc.vector.tensor_mul(inner(sc0), inner(sc0), inner(negivs))
                        nc.scalar.activation(
                            inner(sc1), inner(sc0), mybir.ActivationFunctionType.Exp)
                        nc.vector.tensor_mul(inner(mk[k]), inner(sc1), shifted(vpl[dy], dx))
                        if k == 0:
                            nc.vector.tensor_copy(inner(wsum), inner(mk[k]))
                        else:
                            nc.vector.tensor_add(inner(wsum), inner(wsum), inner(mk[k]))
                    nc.vector.tensor_scalar(
                        inner(sc0), inner(wsum), 1e-30, None, mybir.AluOpType.max)
                    nc.vector.reciprocal(inner(sc1), inner(sc0))
                    wnorm = ph2.tile([128, NG, R], F32)
                    nc.vector.tensor_mul(inner(wnorm), inner(sc1), inner(notc))

                    # finalize: w_k = mk * wnorm, cast+dup to fp16 ch-interleave,
                    # partition-pre-shift by -dy into wde[k]
                    for k, (dx, dy) in enumerate(OFFSETS):
                        wt = ph2.tile([128, NG, R], F32, tag="wt")
                        nc.vector.tensor_mul(inner(wt), inner(mk[k]), inner(wnorm))
                        w16 = ph2.tile([128, W2], dt16, tag="w16", bufs=2)
                        nc.vector.memset(w16[:], 0.0)
                        wv = w16[:].rearrange("p (g r c) -> p g r c", g=NG, r=R, c=2)
                        nc.vector.tensor_copy(wv[:, :, 1 : R - 1, 0], inner(wt))
                        nc.vector.tensor_copy(wv[:, :, 1 : R - 1, 1], inner(wt))
                        nc.vector.memset(wde[k][:], 0.0)
                        hw2 = (W2 // 2) & ~1
                        for (eng, a, b) in ((nc.sync, 0, hw2), (nc.gpsimd, hw2, W2)):
                            if dy == 0:
                                eng.dma_start(wde[k][:, a:b], w16[:, a:b])
                            elif dy == 1:
                                eng.dma_start(wde[k][1:128, a:b], w16[0:127, a:b])
                            else:
                                eng.dma_start(wde[k][0:127, a:b], w16[1:128, a:b])

            # ---------------- Jacobi iterations ----------------
            # terms grouped by stationary matrix (fewer PE weight reloads):
            # (None = the b term) with M0, then dy=-1 taps with Mm, dy=+1 with Mp
            terms = [(None, 0)]
            for k, (dx, dy) in enumerate(OFFSETS):
                if dy == 0:
                    terms.append((k, 0))
            for k, (dx, dy) in enumerate(OFFSETS):
                if dy == -1:
                    terms.append((k, 2))
            for k, (dx, dy) in enumerate(OFFSETS):
                if dy == 1:
                    terms.append((k, 1))

            def xview(a, b):
                return x16[:, PADE + a : PADE + b].rearrange(
                    "p (g r c) -> p g r c", g=(b - a) // R2, r=R, c=2)

            with (
                tc.tile_pool(name="qp", bufs=1) as qp,
                tc.tile_pool(name="pp", bufs=1, space="PSUM") as pp,
            ):
                psets = []
                qtiles = []
                for si, (g0, g1) in enumerate(sets):
                    sw = (g1 - g0) * R2
                    nbank = -(-sw // 512)
                    psets.append(pp.tile([128, nbank * 512], F32, name=f"ps{si}",
                                         tag=f"ps{si}"))
                    row = []
                    for k in range(8):
                        qt = qp.tile([128, sw], dt16, name=f"qt{si}_{k}",
                                     tag=f"qt{si}_{k}")
                        nc.vector.memset(qt[:], 0.0)
                        row.append(qt)
                    qtiles.append(row)
                # per-dy partition range for the tap multiplies: only the
                # partitions the shift matrix actually consumes, so taps
                # depend only on the guard DMAs they truly need
                PRANGE = {0: (0, 127), -1: (0, 127), 1: (0, 128)}
                for it in range(p.n_iters):
                    for si, (g0, g1) in enumerate(sets):
                        lo2, hi2 = g0 * R2, g1 * R2
                        sw = hi2 - lo2
                        ps = psets[si]
                        qts = {}
                        korder = [k for k, (dx, dy) in enumerate(OFFSETS) if dy == 0]
                        korder += [k for k, (dx, dy) in enumerate(OFFSETS) if dy == -1]
                        korder += [k for k, (dx, dy) in enumerate(OFFSETS) if dy == 1]
                        for k in korder:
                            dx, dy = OFFSETS[k]
                            qt = qtiles[si][k]
                            pa, pb = PRANGE[dy]
                            nc.vector.tensor_mul(
                                qt[pa:pb],
                                wde[k][pa:pb, lo2:hi2],
                                x16[pa:pb, PADE + lo2 + 2 * dx : PADE + hi2 + 2 * dx],
                            )
                            qts[k] = qt
                        chs = _chunks(sw)
                        for ti, (k, mi) in enumerate(terms):
                            for (co, cs) in chs:
                                rhs = (b16[:, PADE + lo2 + co : PADE + lo2 + co + cs]
                                       if k is None else qts[k][:, co : co + cs])
                                nc.tensor.matmul(
                                    ps[:, co : co + cs], mats[:, mi, :], rhs,
                                    start=(ti == 0), stop=(ti == len(terms) - 1))
                        # evacuate all real rows of the set (guard rows skipped,
                        # so reads that slop across set boundaries stay dep-free)
                        pv = ps[:, :sw].rearrange(
                            "p (g r c) -> p g r c", g=g1 - g0, r=R, c=2)
                        nc.scalar.copy(
                            xview(lo2, hi2)[:, :, 1 : R - 1, :],
                            pv[:, :, 1 : R - 1, :])
                        # guard partition refresh for boundaries [max(g0,1), g1)
                        j0, j1 = max(g0, 1), g1
                        if j1 > j0:
                            nc.sync.dma_start(
                                xview(j0 * R2, j1 * R2)[0:1, :, 1 : R - 1, :],
                                xview((j0 - 1) * R2, (j1 - 1) * R2)[126:127, :, 1 : R - 1, :])
                            nc.gpsimd.dma_start(
                                xview((j0 - 1) * R2, (j1 - 1) * R2)[127:128, :, 1 : R - 1, :],
                                xview(j0 * R2, j1 * R2)[1:2, :, 1 : R - 1, :])

                    if (it + 1) % T == 0 and (it + 1) < p.n_iters:
                        xr = x16[:, PADE : PADE + W2].rearrange(
                            "p (g r c) -> p g r c", g=NG, r=R, c=2)
                        nc.sync.dma_start(xbnd[:, 0], xr[:, :, T + 1 : 2 * T + 1, :])
                        nc.sync.dma_start(xbnd[:, 1], xr[:, :, RPC + 1 : RPC + T + 1, :])
                        nc.gpsimd.collective_compute(
                            "AllGather",
                            mybir.AluOpType.bypass,
                            replica_groups=[list(range(p.ncores))],
                            ins=[xbnd.opt()],
                            outs=[xgath.opt()],
                        )
                        for r in range(p.ncores):
                            nc.sync.dma_start(xg_sb[:, r], xgath[r])
                        gtop = xr[:, :, 1 : T + 1, :]
                        gbot = xr[:, :, RPC + T + 1 : RPC + 2 * T + 1, :]
                        for reg, dst, ucol in ((1, gtop, 0), (0, gbot, 8)):
                            nc.vector.tensor_scalar_mul(
                                dst, xg_sb[:, 0, reg], uhot[:, ucol : ucol + 1])
                            for r in range(1, p.ncores):
                                nc.vector.scalar_tensor_tensor(
                                    dst, xg_sb[:, r, reg],
                                    uhot[:, ucol + r : ucol + r + 1], dst,
                                    mybir.AluOpType.mult, mybir.AluOpType.add)

            # ---------------- output: yiq2rgb on owned rows ----------------
            with tc.tile_pool(name="ph3", bufs=1) as ph3:
                o32 = ph3.tile([128, NG, RPC, 3], F32)
                t3a = ph3.tile([128, NG, RPC], F32)
                xv = x16[:, PADE : PADE + W2].rearrange(
                    "p (g r c) -> p g r c", g=NG, r=R, c=2)
                xi = xv[:, :, T + 1 : T + 1 + RPC, 0]
                xq = xv[:, :, T + 1 : T + 1 + RPC, 1]
                yo = y32[:, :, T + 1 : T + 1 + RPC]
                for ch in range(3):
                    cy, ci, cq = YIQ2RGB[ch]
                    nc.vector.scalar_tensor_tensor(
                        t3a[:], xi, ci, yo, mybir.AluOpType.mult, mybir.AluOpType.add)
                    nc.vector.scalar_tensor_tensor(
                        t3a[:], xq, cq, t3a[:], mybir.AluOpType.mult, mybir.AluOpType.add)
                    nc.vector.tensor_scalar(
                        t3a[:], t3a[:], 0.0, 1.0, mybir.AluOpType.max, mybir.AluOpType.min)
                    nc.vector.tensor_scalar_mul(o32[:, :, :, ch], t3a[:], 255.0)
                nc.sync.dma_start(out_d[:], o32[:])

    nc.compile()
    return nc


# ---------------------------------------------------------------------------
# host-side sharding / assembly
# ---------------------------------------------------------------------------

def host_inputs(p: Params, gray: np.ndarray, appx: np.ndarray):
    """Build the per-core input maps."""
    H, W, T, NG, R, RPC = p.H, p.W, p.T, p.NG, p.R, p.rpc
    colw = p.cpg * NG + 2  # padded column index range: col -1 .. cpg*NG
    rpad = T + 1

    def padimg(img):
        return np.pad(
            img.astype(np.float32),
            ((rpad, R), (1, colw - 1 - W), (0, 0)),
        )

    gpad = padimg(gray)
    apad = padimg(appx)
    vpad = np.pad(np.ones((H, W), np.float32), ((rpad, R), (1, colw - 1 - W)))

    M = np.zeros((3, 128, 128), p.np16)
    for pp_ in range(1, 127):
        M[0, pp_, pp_] = 1
        M[1, pp_ + 1, pp_] = 1
        M[2, pp_ - 1, pp_] = 1

    in_maps = []
    for c in range(p.ncores):
        r0 = RPC * c
        gT = np.empty((NG, 128, R, 3), np.float32)
        aT = np.empty((NG, 128, R, 3), np.float32)
        vT = np.empty((NG, 128, R), np.float32)
        for g in range(NG):
            c0 = p.cpg * g
            gT[g] = gpad[r0 : r0 + R, c0 : c0 + 128].transpose(1, 0, 2)
            aT[g] = apad[r0 : r0 + R, c0 : c0 + 128].transpose(1, 0, 2)
            vT[g] = vpad[r0 : r0 + R, c0 : c0 + 128].T
        uhot = np.zeros((128, 16), np.float32)
        uhot[:, (c - 1) % p.ncores] = 1
        uhot[:, 8 + (c + 1) % p.ncores] = 1
        in_maps.append(
            {"gray": gT, "appx": aT, "vmask": vT, "mats": M, "uhot": uhot})
    return in_maps


def assemble(p: Params, results):
    """results: list (per core) of {"out": [128, NG, RPC, 3]} -> [H, W, 3]."""
    img = np.zeros((p.H, p.W, 3), np.float32)
    for c in range(p.ncores):
        o = np.asarray(results[c]["out"])
        r0 = p.rpc * c
        for g in range(p.NG):
            ncols = min(p.cpg, p.W - p.cpg * g)
            img[r0 : r0 + p.rpc, p.cpg * g : p.cpg * g + ncols] = (
                o[1 : 1 + ncols, g].transpose(1, 0, 2))
    return img


# ---------------------------------------------------------------------------
# entry point
# ---------------------------------------------------------------------------

_CACHE = {}


def _get_program(p: Params):
    if p not in _CACHE:
        _CACHE[p] = build(p)
    return _CACHE[p]


def kernel(gray_rgb: np.ndarray, appendix_rgb: np.ndarray) -> np.ndarray:
    from concourse.bass_utils import run_bass_kernel_spmd

    p = Params()
    nc = _get_program(p)
    in_maps = host_inputs(p, np.asarray(gray_rgb), np.asarray(appendix_rgb))
    res = run_bass_kernel_spmd(nc, in_maps, list(range(p.ncores)))
    return assemble(p, res.results)

